# revision 45
# baseline (speedup 1.0000x reference)
"""Trainium2 Bass kernel for PointTPS: warp 120000 query points with a
1024-control-point thin-plate-spline.

Math: out[b,:] = pts[b,:] + sum_a W[a,:] * 0.5*l2(a,b)*ln(l2(a,b))
               + W[1024,:] + W[1025,:]*x_b + W[1026,:]*y_b

Device scheme (per core, 15360 points, 8 chunks of 128 control points):
  - l2 has rank 4: l2+eps = sum_k g_k(a)*f_k(b), g=[|a'|^2,-2a'x,-2a'y,1],
    f=[1,b'x,b'y,|b'|^2+eps] (coords centered at 50). fp32 matmuls run at
    1/4 rate on TRN2, so MM1 uses a triple-bf16 limb decomposition:
    g=g1+g2+g3, f=f1+f2+f3, keeping the 6 dominant limb products as a
    single K=24 bf16 matmul (full fp32-level precision, 4x faster).
  - ACT computes LN = ln(l2/L2_0) straight from PSUM (scale folded into
    the activation multiplier). Centering the log shrinks the fp32
    cancellation error of the W-sums ~10x; the removed 0.5*ln(L2_0)*
    sum_a(W*l2) term is affine in b and folded back on the host.
  - The elementwise multiply l2*ln never happens on the full matrix:
    z_j = sum_k f_k * S_kj with S_kj(b) = sum_a 0.5*W*g_k * LN(a,b).
    The S-contraction runs in SWAP orientation: the LN block is the
    STATIONARY operand and the host-precomputed H [128,8] moves, so each
    fp32 matmul's output is only 8 columns wide (output-width-priced) —
    64 near-free matmuls per tile instead of 16 at 512-wide. S lands
    transposed [points, (block,k,j)]; DVE then does one small multiply
    T = S*F_T, a strided free-dim reduce over k, and an add.
  - Affine+identity+log-centering corrections ride tiny bf16 limb
    matmuls into spare columns of the same PSUM bank; the output leaves
    point-major and the host transposes it back.

Sharding: pure data parallelism over points; 8 cores run the identical
program on different 15360-point slices (padded from 120000).
"""

import os
import sys

import numpy as np

if "/opt/trn_rl_repo" not in sys.path:
    sys.path.insert(0, "/opt/trn_rl_repo")

N_FULL = 120000
N_CORES = 8
TILE = 1536
TILES_PER_CORE = 10
N_CORE = TILE * TILES_PER_CORE            # 15360
N_KPS = 1024
CHUNKS = 8                                 # 1024 / 128
BLOCKS = TILE // 128                       # 128-pt blocks per tile
HALVES = TILE // 512                       # 512-wide matmul slices
STW = BLOCKS * 8                           # St data cols
STA = BLOCKS * 2                           # St affine cols
C0 = np.float64(50.0)                      # coordinate centering
EPS = np.float64(1e-2)                     # keeps ln argument > 0
L2_0 = np.float64(7000.0)                  # log centering constant

# triple-bf16 limb pairing: l2 ~= sum of g_i * f_j over these 6 blocks
F_BLOCKS = [0, 1, 0, 2, 1, 0]              # f limb index per 4-row block
G_BLOCKS = [0, 0, 1, 0, 1, 2]              # g limb index per 4-row block

_CACHE = {}


# Column-tiled S-accumulation (concurrent PE sub-tiles) is implemented but
# its has_written/bank-clear interaction is not yet correct on hardware;
# default stays on the verified plain path.
COLTILE = bool(int(os.environ.get("KERNEL_COLTILE", "0")))


def _build_program():
    import concourse.bass as bass
    import concourse.tile as tile
    from concourse import mybir
    from concourse.tile_rust import add_dep_helper

    f32 = mybir.dt.float32
    bf16 = mybir.dt.bfloat16
    Ln = mybir.ActivationFunctionType.Ln

    nc = bass.Bass("TRN2", target_bir_lowering=False, debug=False,
                   num_devices=N_CORES)

    # per-point operands
    fstk = nc.dram_tensor("fstk", [24, N_CORE], bf16, kind="ExternalInput")
    # transposed F: ftr[p, t*64 + 8b + k*2 + j] = f_k(point t*1024+128b+p)
    ftr = nc.dram_tensor("ftr", [128, STW * TILES_PER_CORE], f32,
                         kind="ExternalInput")
    # fp32 constants: cols 0:64 = H
    cmat = nc.dram_tensor("cmat", [128, 66], f32, kind="ExternalInput")
    # bf16 constants: cols 0:1024 = g limb stack, cols 1024:1026 = aff24
    gcon = nc.dram_tensor("gcon", [24, 1026], bf16, kind="ExternalInput")
    # transposed output: outp[p, t*16 + 2b + j] = out_j(point t*1024+128b+p)
    outp = nc.dram_tensor("outp", [128, STA * TILES_PER_CORE], f32,
                          kind="ExternalOutput")

    with tile.TileContext(nc) as tc:
        with (
            tc.tile_pool(name="const", bufs=1) as constp,
            tc.tile_pool(name="fin", bufs=3) as finp,
            tc.tile_pool(name="lnp", bufs=16) as lnp,
            tc.tile_pool(name="work", bufs=3) as workp,
            tc.tile_pool(name="outs", bufs=3) as outsp,
            tc.tile_pool(name="l2p", bufs=2, space="PSUM") as l2p,
            tc.tile_pool(name="sp", bufs=2, space="PSUM") as sp,
        ):
            ct = constp.tile([128, 66], f32)
            nc.sync.dma_start(ct[:], cmat[:])
            h_t = ct[:, 0:64]
            gt = constp.tile([24, 1026], bf16)
            nc.sync.dma_start(gt[:], gcon[:])
            g_t = gt[:, 0:N_KPS]
            aff_t = gt[:, 1024:1026]

            # gate matmuls: PE observes each const DMA semaphore once so no
            # later matmul needs more than one sync wait (walrus limit).
            # They scribble into an St-pool slot; every element of a real
            # St tile is freshly written each tile, so no cleanup needed.
            sg = sp.tile([128, STW + STA], f32, tag="st", name="sg")
            nc.tensor.matmul(sg[0:64, 0:64], ct[0:4, 0:64], ct[0:4, 0:64],
                             start=True, stop=True)
            nc.tensor.matmul(sg[0:80, 0:80], gt[:, 0:80], gt[:, 0:80],
                             start=True, stop=True)

            inv_l20 = float(1.0 / L2_0)

            # warmup activation: pulls the ~2.7us Ln table load into the
            # pipeline-fill phase instead of serializing it before the
            # first real log (input is the const tile, H cols, |.|>0
            # irrelevant — output discarded)
            warm = constp.tile([1, 2], f32, name="warm")
            nc.scalar.activation(warm[:], ct[0:1, 0:2], Ln)

            for t in range(TILES_PER_CORE):
                col = slice(t * TILE, (t + 1) * TILE)
                fs = finp.tile([24, TILE], bf16, tag="fs")
                nc.sync.dma_start(fs[:], fstk[:, col])
                ft = finp.tile([128, STW], f32, tag="ft")
                nc.sync.dma_start(ft[:], ftr[:, t * STW:(t + 1) * STW])

                lns = []
                for c in range(CHUNKS):
                    l2t = l2p.tile([128, TILE], f32, tag="l2",
                                   name=f"l2_{t}_{c}")
                    for h in range(HALVES):
                        hs = slice(h * 512, (h + 1) * 512)
                        nc.tensor.matmul(
                            l2t[:, hs],
                            g_t[:, c * 128:(c + 1) * 128],
                            fs[:, hs],
                            start=True, stop=True,
                        )
                    ln_t = lnp.tile([128, TILE], f32, tag="ln",
                                    name=f"ln_{t}_{c}")
                    nc.scalar.activation(ln_t[:], l2t[:], Ln, scale=inv_l20)
                    lns.append(ln_t)

                # Swap-orientation S-contraction: LN block is STATIONARY,
                # H moves -> out St[p, 8b+k*2+j] is only 8 columns wide per
                # matmul (near-free vs 512-wide). Accumulate over chunks.
                # Order is load-bearing: the single start=True (c=0, b=0)
                # clears the whole bank's has_written bits and must issue
                # first; everything else is start=False (first write per
                # element overwrites, later ones accumulate).
                st = sp.tile([128, STW + STA], f32, tag="st", name=f"st_{t}")
                prev_mm = None
                for c in range(CHUNKS):
                    for b in range(BLOCKS):
                        mm = nc.tensor.matmul(
                            st[:, 8 * b:8 * b + 8],
                            lns[c][:, 128 * b:128 * (b + 1)],
                            h_t[:, c * 8:(c + 1) * 8],
                            start=(c == 0 and b == 0), stop=False,
                            skip_group_check=True,
                        )
                        if prev_mm is not None:
                            add_dep_helper(mm.ins, prev_mm, sync=False,
                                           reason="st-order")
                        prev_mm = mm.ins
                # affine + identity + corrections: bf16 limb matmuls into
                # cols 64:80 (same bank; after all S writes so their
                # start=True bit-clears can't corrupt the accumulation)
                for b in range(BLOCKS):
                    mm = nc.tensor.matmul(
                        st[:, STW + 2 * b:STW + 2 * b + 2],
                        fs[:, 128 * b:128 * (b + 1)],
                        aff_t[:],
                        start=True, stop=(b == BLOCKS - 1),
                        skip_group_check=True,
                    )
                    add_dep_helper(mm.ins, prev_mm, sync=False,
                                   reason="aff-after-st")
                    prev_mm = mm.ins

                # T = S * F_T (one small DVE multiply), then reduce over k
                # (innermost, stride 2) per j, and add the affine part.
                t_t = workp.tile([128, STW], f32, tag="T", name=f"T_{t}")
                nc.vector.tensor_mul(t_t[:], st[:, 0:STW], ft[:])
                zt = outsp.tile([128, STA], f32, tag="zt", name=f"zt_{t}")
                for j in range(2):
                    zz = workp.tile([128, BLOCKS], f32, tag=f"zz{j}",
                                    name=f"zz{j}_{t}")
                    red_in = t_t[:, j:STW:2].rearrange(
                        "p (b k) -> p b k", k=4)
                    nc.vector.tensor_reduce(
                        zz[:], red_in, axis=mybir.AxisListType.X,
                        op=mybir.AluOpType.add)
                    nc.vector.tensor_add(
                        zt[:, j:STA:2], zz[:], st[:, STW + j:STW + STA:2])
                nc.sync.dma_start(outp[:, t * STA:(t + 1) * STA], zt[:])

    _split_multi_waits(nc)
    return nc


def _split_multi_waits(nc):
    """This toolchain's walrus encodes at most ONE sync wait per hardware
    instruction (setupSyncWait asserts). Tile emits multi-wait instructions
    for pool-slot reuse; split the extras onto preceding same-engine nops
    (strict FIFO queues make this semantically identical)."""
    from concourse import mybir

    n_split = 0
    for f in nc.m.functions:
        for b in f.blocks:
            new_insts = []
            for ins in b.instructions:
                si = ins.sync_info
                if si is not None and si.on_wait and len(si.on_wait) > 1:
                    waits = list(si.on_wait)
                    for w in waits[:-1]:
                        nop = mybir.InstNoOp(
                            name=f"waitnop-{n_split}-{len(new_insts)}",
                            engine=ins.engine,
                            bass_nofuse=True,
                            sync_info=mybir.SyncInfo(on_wait=[w], on_update=[]),
                        )
                        new_insts.append(nop)
                    ins.sync_info = mybir.SyncInfo(
                        on_wait=[waits[-1]],
                        on_update=list(si.on_update or []),
                    )
                    n_split += 1
                new_insts.append(ins)
            b.instructions = new_insts
    return n_split


def _bf16_limbs(x, n=3):
    """Split float64 array into n bf16 limbs summing to ~fp32 precision."""
    import ml_dtypes
    limbs = []
    r = np.asarray(x, np.float64).copy()
    for _ in range(n):
        l = r.astype(np.float32).astype(ml_dtypes.bfloat16)
        limbs.append(l)
        r = r - l.astype(np.float64)
    return limbs


def _pack_inputs(pts, kps_a, W):
    import ml_dtypes
    bf = ml_dtypes.bfloat16

    pts = np.asarray(pts, np.float32)
    kps_a = np.asarray(kps_a, np.float32)
    W = np.asarray(W, np.float32)

    a = kps_a.astype(np.float64) - C0
    aa = a[:, 0] ** 2 + a[:, 1] ** 2

    # g rows (float64): [|a'|^2, -2a'x, -2a'y, 1]
    g = np.stack([aa, -2.0 * a[:, 0], -2.0 * a[:, 1], np.ones(N_KPS)], axis=0)
    g_l = _bf16_limbs(g)                                  # 3 x [4, N_KPS]

    # H[m, c*8 + k*2 + j] = 0.5 * W[c*128+m, j] * g[k, c*128+m]  (fp32)
    h = np.zeros((128, 8 * CHUNKS), np.float64)
    for c in range(CHUNKS):
        rows = slice(c * 128, (c + 1) * 128)
        blk = 0.5 * W[rows].astype(np.float64)[:, None, :] * \
            g[:, rows].T[:, :, None]
        h[:, c * 8:(c + 1) * 8] = blk.reshape(128, 8)
    h = h.astype(np.float32)

    # selector: K=104, rows 32g + k*2 + j -> col j. Under COLTILE=0 only
    # group 0 holds S data (rows 64:66 hold z — must stay unselected).
    sel = np.zeros((104, 2), np.float32)
    for grp in (range(4) if COLTILE else (0,)):
        for k in range(4):
            sel[32 * grp + k * 2 + 0, 0] = 1.0
            sel[32 * grp + k * 2 + 1, 1] = 1.0

    # affine + identity + log-centering corrections (float64)
    w64 = W.astype(np.float64)
    L = np.log(L2_0)
    sumW = w64[:N_KPS].sum(0)
    sumWa = (w64[:N_KPS, None, :] * a[:, :, None]).sum(0)
    sumWaa = (w64[:N_KPS] * aa[:, None]).sum(0)

    A = np.zeros((4, 2), np.float64)        # rows [1, x, y, r2-placeholder]
    A[0] = w64[1024] + C0 * (w64[1025] + w64[1026]) + C0 + 0.5 * L * sumWaa
    A[1] = w64[1025] + np.array([1.0, 0.0]) - L * sumWa[0]
    A[2] = w64[1026] + np.array([0.0, 1.0]) - L * sumWa[1]
    # the (bb+eps)*sumW part of the centering correction rides f row 3
    A[3] = 0.5 * L * sumW
    A_l = _bf16_limbs(A)                                  # 3 x [4, 2]

    # aff24: A limb per block, paired with the f limb blocks
    aff24 = np.zeros((24, 2), np.float64)
    for b_i, (fi, gi) in enumerate(zip(F_BLOCKS, G_BLOCKS)):
        aff24[b_i * 4:(b_i + 1) * 4] = A_l[gi].astype(np.float64)
    aff24 = aff24.astype(np.float32).astype(bf)

    cm = np.zeros((128, 66), np.float32)
    cm[:, 0:64] = h
    cm[0:104, 64:66] = sel

    gcon = np.zeros((24, 1026), np.float32)
    for b_i, gi in enumerate(G_BLOCKS):
        gcon[b_i * 4:(b_i + 1) * 4, 0:N_KPS] = g_l[gi].astype(np.float32)
    gcon[:, 1024:1026] = aff24.astype(np.float32)
    gcon = gcon.astype(bf)

    # per-point features, padded
    n_pad = N_CORE * N_CORES
    b = np.zeros((n_pad, 2), np.float64)
    b[:N_FULL] = pts.astype(np.float64) - C0
    bb = b[:, 0] ** 2 + b[:, 1] ** 2
    f = np.stack([np.ones(n_pad), b[:, 0], b[:, 1], bb + EPS], axis=0)
    f_l = _bf16_limbs(f)                                  # 3 x [4, n_pad]

    fstk = np.zeros((24, n_pad), np.float32)
    for b_i, fi in enumerate(F_BLOCKS):
        fstk[b_i * 4:(b_i + 1) * 4] = f_l[fi].astype(np.float32)
    fstk = fstk.astype(bf)

    # transposed F: ftr[p, t*64 + 8b + k*2 + j] = f_k(pt t*1024+128b+p)
    f32v = f.astype(np.float32)                           # [4, n_pad]
    # view per core below; build globally: [4, cores, t, b, p] -> ...
    fview = f32v.reshape(4, N_CORES, TILES_PER_CORE, BLOCKS, 128)
    # target per core: [128(p), t, b, k, j]
    ftr_all = np.repeat(
        fview.transpose(1, 4, 2, 3, 0),                  # [core, p, t, b, k]
        2, axis=4).reshape(N_CORES, 128, TILES_PER_CORE, BLOCKS, 4, 2)
    # fix the repeat: we need k dup'd over j, i.e. [..., k, j]
    ftr_all = np.ascontiguousarray(
        ftr_all.reshape(N_CORES, 128, TILES_PER_CORE * STW))

    in_maps = []
    for i in range(N_CORES):
        cs = slice(i * N_CORE, (i + 1) * N_CORE)
        in_maps.append({
            "fstk": np.ascontiguousarray(fstk[:, cs]),
            "ftr": ftr_all[i],
            "cmat": cm,
            "gcon": gcon,
        })
    return in_maps


def kernel(pts, kps_a, W):
    from concourse.bass_utils import run_bass_kernel_spmd

    if "nc" not in _CACHE:
        _CACHE["nc"] = _build_program()
    nc = _CACHE["nc"]

    in_maps = _pack_inputs(pts, kps_a, W)

    trace = bool(int(os.environ.get("KERNEL_TRACE", "0")))
    res = run_bass_kernel_spmd(
        nc, in_maps, core_ids=list(range(N_CORES)), trace=trace,
    )
    _CACHE["last_result"] = res

    out = np.empty((N_CORE * N_CORES, 2), np.float32)
    for i in range(N_CORES):
        o = res.results[i]["outp"].reshape(128, TILES_PER_CORE, BLOCKS, 2)
        out[i * N_CORE:(i + 1) * N_CORE] = \
            o.transpose(1, 2, 0, 3).reshape(N_CORE, 2)
    return out[:N_FULL]
